# revision 1
# baseline (speedup 1.0000x reference)
"""Trainium2 Bass kernel for nn_Attention_15556371546220 (Enformer-style
relative-position attention, B=1 L=4096 C=768 H=4 DK=64 DV=192 POSF=64).

Sharding: 8 cores = 4 heads x 2 query-blocks of 2048. Each core computes its
head's K/V over the full sequence, Q over its query block, full attention with
the relative-shift positional term, and a partial output projection
(row-parallel over the head's 192 value dims). Host gathers: sums the 4 head
partials per query block and adds the output bias.

Relative shift: shifted[i,j] = (q_i/8 + rpb) . pk[j - i + 4095] is computed as
a per-query-tile matmul U[p,m] = y_p . pk[wstart+m] (width 4223), stored to a
DRAM scratch (pitch 4224, fp16) and read back with a skewed strided access
pattern (row stride 4223) which realizes U[p, j+127-p] -- the exact shift.
"""
import sys
if "/opt/trn_rl_repo" not in sys.path:
    sys.path.insert(0, "/opt/trn_rl_repo")

import numpy as np
import ml_dtypes

import concourse.bass as bass
import concourse.bacc as bacc
import concourse.mybir as mybir
import concourse.tile as tile
from concourse.bass_utils import run_bass_kernel_spmd

F32 = mybir.dt.float32
BF16 = mybir.dt.bfloat16
FP16 = mybir.dt.float16
AX = mybir.AxisListType
ALU = mybir.AluOpType
ACT = mybir.ActivationFunctionType

B, L, C = 1, 4096, 768
H, DK, DV = 4, 64, 192
POSF = 64
NQ = 2048          # queries per core (one of two blocks)
NT = 16            # query tiles of 128 per core
UW = 4223          # U window width per query tile
UP = 4224          # U row pitch in DRAM scratch
PKW = 6144         # per-core pos-key window (covers all 16 tiles)

_nc_cache = {}

import os
_LEVEL = int(os.environ.get("KLEVEL", "5"))  # 1=projA 2=+U/content/exp 3=+transpose 4=+oT 5=full


def _build_nc():
    nc = bacc.Bacc()

    xt_in = nc.declare_dram_parameter("xt", (C, L), FP16, isOutput=False)
    xq_in = nc.declare_dram_parameter("xq", (C, NQ), FP16, isOutput=False)
    wq_in = nc.declare_dram_parameter("wq", (C, DK), FP16, isOutput=False)
    wk_in = nc.declare_dram_parameter("wk", (C, DK), FP16, isOutput=False)
    wv_in = nc.declare_dram_parameter("wv", (C, DV), FP16, isOutput=False)
    wpos_in = nc.declare_dram_parameter("wpos", (POSF, DK), FP16, isOutput=False)
    post_in = nc.declare_dram_parameter("post", (POSF, PKW), FP16, isOutput=False)
    wout_in = nc.declare_dram_parameter("wout", (DV, C), FP16, isOutput=False)
    rcb_in = nc.declare_dram_parameter("rcb", (DK, 1), F32, isOutput=False)
    rpb_in = nc.declare_dram_parameter("rpb", (DK, 1), F32, isOutput=False)
    ident_in = nc.declare_dram_parameter("ident", (128, 128), BF16, isOutput=False)
    out_dram = nc.declare_dram_parameter("out", (NQ, C), F32, isOutput=True)

    with tile.TileContext(nc) as tc:
        with (
            tc.tile_pool(name="const", bufs=1) as cpool,
            tc.tile_pool(name="res", bufs=1) as rpool,
            tc.tile_pool(name="udram", bufs=3, space="DRAM") as dpool,
        ):
            # ---------- constants ----------
            wq_sb = cpool.tile([128, 6, DK], FP16)
            nc.gpsimd.dma_start(wq_sb[:], wq_in.rearrange("(cc p) d -> p cc d", p=128))
            wk_sb = cpool.tile([128, 6, DK], FP16)
            nc.gpsimd.dma_start(wk_sb[:], wk_in.rearrange("(cc p) d -> p cc d", p=128))
            wv_sb = cpool.tile([128, 6, DV], FP16)
            nc.gpsimd.dma_start(wv_sb[:], wv_in.rearrange("(cc p) d -> p cc d", p=128))
            wpos_sb = cpool.tile([POSF, DK], FP16)
            nc.gpsimd.dma_start(wpos_sb[:], wpos_in[:])
            wout1_sb = cpool.tile([128, C], FP16)
            nc.gpsimd.dma_start(wout1_sb[:], wout_in[0:128, :])
            wout2_sb = cpool.tile([64, C], FP16)
            nc.gpsimd.dma_start(wout2_sb[:], wout_in[128:192, :])
            rcb_sb = cpool.tile([DK, 1], F32)
            nc.gpsimd.dma_start(rcb_sb[:], rcb_in[:])
            rpb_sb = cpool.tile([DK, 1], F32)
            nc.gpsimd.dma_start(rpb_sb[:], rpb_in[:])
            ident_sb = cpool.tile([128, 128], BF16)
            nc.gpsimd.dma_start(ident_sb[:], ident_in[:])

            # ---------- residents ----------
            kt_sb = rpool.tile([DK, L], FP16)          # K^T
            v1_sb = rpool.tile([128, 32, 128], FP16)   # V[:, :128] per key block
            v2_sb = rpool.tile([128, 32, 64], FP16)    # V[:, 128:192]
            pkt_sb = rpool.tile([DK, PKW], FP16)       # pos_k^T window
            qct_sb = rpool.tile([DK, NQ], FP16)        # (Q/8 + rcb)^T
            qpt_sb = rpool.tile([DK, NQ], FP16)        # (Q/8 + rpb)^T

            # ---------- phase A: projections ----------
            with (
                tc.tile_pool(name="xa", bufs=8) as xpool,
                tc.tile_pool(name="psA", bufs=2, space="PSUM") as psA,
                tc.tile_pool(name="psV", bufs=2, space="PSUM") as psV,
                tc.tile_pool(name="psQ", bufs=2, space="PSUM") as psQ,
            ):
                for kc in range(8):
                    xts = []
                    for cc in range(6):
                        xt_t = xpool.tile([128, 512], FP16, tag="xs")
                        nc.sync.dma_start(
                            xt_t[:], xt_in[cc * 128:(cc + 1) * 128,
                                           kc * 512:(kc + 1) * 512])
                        xts.append(xt_t)
                    kt_ps = psA.tile([DK, 512], F32, tag="psa")
                    for cc in range(6):
                        nc.tensor.matmul(kt_ps[:], wk_sb[:, cc, :], xts[cc][:],
                                         start=(cc == 0), stop=(cc == 5))
                    nc.scalar.copy(kt_sb[:, kc * 512:(kc + 1) * 512], kt_ps[:])
                    for sub in range(4):
                        kb = kc * 4 + sub
                        v_ps = psV.tile([128, DV], F32, tag="psv")
                        for cc in range(6):
                            nc.tensor.matmul(
                                v_ps[:], xts[cc][:, sub * 128:(sub + 1) * 128],
                                wv_sb[:, cc, :], start=(cc == 0), stop=(cc == 5))
                        nc.vector.tensor_copy(v1_sb[:, kb, :], v_ps[:, 0:128])
                        nc.vector.tensor_copy(v2_sb[:, kb, :], v_ps[:, 128:192])

                for qc in range(4):
                    q_ps = psQ.tile([DK, 512], F32, tag="psq")
                    for cc in range(6):
                        xq_t = xpool.tile([128, 512], FP16, tag="xs")
                        nc.sync.dma_start(
                            xq_t[:], xq_in[cc * 128:(cc + 1) * 128,
                                           qc * 512:(qc + 1) * 512])
                        nc.tensor.matmul(q_ps[:], wq_sb[:, cc, :], xq_t[:],
                                         start=(cc == 0), stop=(cc == 5))
                    nc.scalar.activation(qct_sb[:, qc * 512:(qc + 1) * 512],
                                         q_ps[:], ACT.Identity,
                                         bias=rcb_sb[:], scale=0.125)
                    nc.scalar.activation(qpt_sb[:, qc * 512:(qc + 1) * 512],
                                         q_ps[:], ACT.Identity,
                                         bias=rpb_sb[:], scale=0.125)

                for mc in range(12):
                    po_t = xpool.tile([POSF, 512], FP16, tag="po")
                    nc.sync.dma_start(
                        po_t[:], post_in[:, mc * 512:(mc + 1) * 512])
                    pk_ps = psA.tile([DK, 512], F32, tag="psa")
                    nc.tensor.matmul(pk_ps[:], wpos_sb[:], po_t[:],
                                     start=True, stop=True)
                    nc.scalar.copy(pkt_sb[:, mc * 512:(mc + 1) * 512], pk_ps[:])

            # ---------- phase B: attention ----------
            with (
                tc.tile_pool(name="ub", bufs=2) as upool,
                tc.tile_pool(name="usk", bufs=4) as uskpool,
                tc.tile_pool(name="at", bufs=3) as apool,
                tc.tile_pool(name="att", bufs=2) as atpool,
                tc.tile_pool(name="zz", bufs=10) as zpool,
                tc.tile_pool(name="ot", bufs=2) as opool,
                tc.tile_pool(name="fin", bufs=2) as fpool,
                tc.tile_pool(name="psU", bufs=2, space="PSUM") as psU,
                tc.tile_pool(name="psC", bufs=2, space="PSUM") as psC,
                tc.tile_pool(name="psT", bufs=1, space="PSUM") as psT,
                tc.tile_pool(name="psO", bufs=1, space="PSUM") as psO,
                tc.tile_pool(name="psP", bufs=1, space="PSUM") as psP,
            ):
                def emit_u(qt):
                    ws = 1920 - 128 * qt
                    u_sb = upool.tile([128, UP], FP16, tag="u")
                    for uc in range(9):
                        w = 512 if uc < 8 else UW - 8 * 512
                        u_ps = psU.tile([128, 512], F32, tag="psu")
                        nc.tensor.matmul(
                            u_ps[:, 0:w],
                            qpt_sb[:, qt * 128:(qt + 1) * 128],
                            pkt_sb[:, ws + uc * 512: ws + uc * 512 + w],
                            start=True, stop=True)
                        if uc % 2 == 0:
                            nc.vector.tensor_copy(
                                u_sb[:, uc * 512: uc * 512 + w], u_ps[:, 0:w])
                        else:
                            nc.scalar.copy(
                                u_sb[:, uc * 512: uc * 512 + w], u_ps[:, 0:w])
                    u_dr = dpool.tile([128, UP], FP16, tag="udr")
                    nc.sync.dma_start(u_dr[:], u_sb[:])
                    return u_dr[:]

                u_next = emit_u(0)
                for st in range(4 if _LEVEL >= 2 else 0):
                    attnT = atpool.tile([128, 32, 512], BF16, tag="attnT")
                    rz_list = []
                    for qt2 in range(4):
                        qt = st * 4 + qt2
                        u_ap = u_next
                        if qt + 1 < 16:
                            u_next = emit_u(qt + 1)
                        # --- content logits + shifted U + exp, per 512 chunk
                        attn = apool.tile([128, L], BF16, tag="attn")
                        zc = zpool.tile([128, 8], F32, tag="zc")
                        for ch in range(8):
                            usk = uskpool.tile([128, 512], FP16, tag="usk")
                            skew = bass.AP(u_ap.tensor,
                                           u_ap.offset + ch * 512 + 127,
                                           [[UW, 128], [1, 512]])
                            nc.gpsimd.dma_start(usk[:], skew)
                            c_ps = psC.tile([128, 512], F32, tag="psc")
                            nc.tensor.matmul(
                                c_ps[:], qct_sb[:, qt * 128:(qt + 1) * 128],
                                kt_sb[:, ch * 512:(ch + 1) * 512],
                                start=True, stop=True)
                            nc.vector.tensor_add(c_ps[:], c_ps[:], usk[:])
                            nc.scalar.activation(
                                attn[:, ch * 512:(ch + 1) * 512], c_ps[:],
                                ACT.Exp, accum_out=zc[:, ch:ch + 1])
                        zs = zpool.tile([128, 1], F32, tag="zs")
                        nc.vector.tensor_reduce(zs[:], zc[:], AX.X, ALU.add)
                        rz = zpool.tile([128, 1], F32, tag="rz")
                        nc.vector.reciprocal(rz[:], zs[:])
                        rz_list.append(rz)
                        if _LEVEL < 3:
                            continue
                        # --- transpose attn into attnT[:, kb, qt2*128:...]
                        for g in range(8):
                            t_ps = psT.tile([128, 512], BF16, tag="pst")
                            for j in range(4):
                                kb = g * 4 + j
                                nc.tensor.transpose(
                                    t_ps[:, j * 128:(j + 1) * 128],
                                    attn[:, kb * 128:(kb + 1) * 128],
                                    ident_sb[:])
                            nc.vector.tensor_copy(
                                attnT[:, g * 4:(g + 1) * 4,
                                      qt2 * 128:(qt2 + 1) * 128],
                                t_ps[:].rearrange("p (j c) -> p j c", j=4))
                    if _LEVEL < 4:
                        continue
                    # --- o^T = V^T @ attn^T over 32 key blocks (512 queries)
                    o1_ps = psO.tile([128, 512], F32, tag="po1")
                    o2_ps = psO.tile([64, 512], F32, tag="po2")
                    for kb in range(32):
                        nc.tensor.matmul(o1_ps[:], v1_sb[:, kb, :],
                                         attnT[:, kb, :],
                                         start=(kb == 0), stop=(kb == 31))
                    for kb in range(32):
                        nc.tensor.matmul(o2_ps[:], v2_sb[:, kb, :],
                                         attnT[:, kb, :],
                                         start=(kb == 0), stop=(kb == 31))
                    o1t = opool.tile([128, 512], BF16, tag="o1")
                    nc.scalar.copy(o1t[:], o1_ps[:])
                    o2t = opool.tile([64, 512], BF16, tag="o2")
                    nc.scalar.copy(o2t[:], o2_ps[:])
                    if _LEVEL < 5:
                        continue
                    # --- projection + normalize + store, per 128-query tile
                    for qt2 in range(4):
                        fin = fpool.tile([128, C], F32, tag="fin")
                        for n0, nw in ((0, 384), (384, 384)):
                            p_ps = psP.tile([128, 384], F32, tag="pp")
                            nc.tensor.matmul(
                                p_ps[:, 0:nw],
                                o1t[:, qt2 * 128:(qt2 + 1) * 128],
                                wout1_sb[:, n0:n0 + nw],
                                start=True, stop=False)
                            nc.tensor.matmul(
                                p_ps[:, 0:nw],
                                o2t[:, qt2 * 128:(qt2 + 1) * 128],
                                wout2_sb[:, n0:n0 + nw],
                                start=False, stop=True)
                            nc.scalar.activation(fin[:, n0:n0 + nw],
                                                 p_ps[:, 0:nw], ACT.Copy,
                                                 scale=rz_list[qt2][:])
                        nc.gpsimd.dma_start(
                            out_dram[(st * 4 + qt2) * 128:
                                     (st * 4 + qt2 + 1) * 128, :], fin[:])

    nc.finalize()
    return nc


def _positions_T():
    feat = POSF // 2
    pow_rate = np.exp(np.log(L + 1) / feat).astype(np.float64)
    pos = np.arange(-L + 1, L, dtype=np.float64)                 # (8191,)
    cw = pow_rate ** np.arange(1, feat + 1, dtype=np.float64) - 1.0
    emb = (cw[None, :] > np.abs(pos)[:, None]).astype(np.float32)
    signed = np.sign(pos)[:, None].astype(np.float32) * emb
    p = np.concatenate([emb, signed], axis=-1)                   # (8191, 64)
    pt = np.zeros((POSF, 2 * L), np.float32)
    pt[:, :2 * L - 1] = p.T
    return pt


def kernel(x, Wq, Wk, Wv, Wpos, Wout, bout, rel_content_bias, rel_pos_bias):
    bf = ml_dtypes.bfloat16
    f16 = np.float16
    if "nc" not in _nc_cache:
        _nc_cache["nc"] = _build_nc()
    nc = _nc_cache["nc"]

    xt = np.ascontiguousarray(x[0].T).astype(f16)                 # (C, L)
    posT = _positions_T()                                        # (64, 8192)
    ident = np.eye(128, dtype=bf)

    in_maps = []
    for c in range(8):
        h, b = c // 2, c % 2
        w0 = 3968 - 2048 * b
        in_maps.append({
            "xt": xt,
            "xq": np.ascontiguousarray(x[0, b * NQ:(b + 1) * NQ].T).astype(f16),
            "wq": Wq[:, h * DK:(h + 1) * DK].astype(f16),
            "wk": Wk[:, h * DK:(h + 1) * DK].astype(f16),
            "wv": Wv[:, h * DV:(h + 1) * DV].astype(f16),
            "wpos": Wpos[:, h * DK:(h + 1) * DK].astype(f16),
            "post": np.ascontiguousarray(
                posT[:, w0 - 1920: w0 - 1920 + PKW]).astype(f16),
            "wout": Wout[h * DV:(h + 1) * DV, :].astype(f16),
            "rcb": np.ascontiguousarray(
                rel_content_bias[0, h, 0][:, None]).astype(np.float32),
            "rpb": np.ascontiguousarray(
                rel_pos_bias[0, h, 0][:, None]).astype(np.float32),
            "ident": ident,
        })

    res = run_bass_kernel_spmd(nc, in_maps, core_ids=list(range(8)))
    globals()["last_results"] = res
    parts = [r["out"] for r in res.results]

    out = np.zeros((L, C), np.float32)
    for b in range(2):
        acc = np.zeros((NQ, C), np.float32)
        for h in range(4):
            acc += parts[h * 2 + b]
        out[b * NQ:(b + 1) * NQ] = acc
    out += bout[None, :].astype(np.float32)
    return out.reshape(1, L, C)



# revision 3
# speedup vs baseline: 1.1243x; 1.1243x over previous
"""Trainium2 Bass kernel for nn_Attention_15556371546220 (Enformer-style
relative-position attention, B=1 L=4096 C=768 H=4 DK=64 DV=192 POSF=64).

Sharding: 8 cores = 4 heads x 2 query-blocks of 2048. Each core computes its
head's K/V over the full sequence, Q over its query block, full attention with
the relative-shift positional term, and a partial output projection
(row-parallel over the head's 192 value dims). Host sums the 4 head partials
per query block and adds the output bias.

v2 structure (vs v1 baseline):
- software-pipelined phase B: per query tile qt the emission order is
  staggered (U emit at qt+2, skew-read at qt+1, content+exp+transpose at qt,
  o-accumulate + out-projection at qt-1) so the tensor engine never drains
  (keeps the PE in its fast p-state).
- softmax denominator via a ones-column appended to V: o_psum[:, 192] = z.
- o computed query-major (one 193-col accumulation over 32 key blocks)
  instead of two dv-major groups; z reciprocal from column 192.
- one batched skew-read DMA per query tile ([128,4096], row stride 4223 over
  a pitch-4224 DRAM scratch realizes the relative shift).
- fp16 partial outputs (host accumulates in fp32).
"""
import sys
if "/opt/trn_rl_repo" not in sys.path:
    sys.path.insert(0, "/opt/trn_rl_repo")

import numpy as np
import ml_dtypes

import concourse.bass as bass
import concourse.bacc as bacc
import concourse.mybir as mybir
import concourse.tile as tile
from concourse.bass_utils import run_bass_kernel_spmd

F32 = mybir.dt.float32
BF16 = mybir.dt.bfloat16
FP16 = mybir.dt.float16
AX = mybir.AxisListType
ALU = mybir.AluOpType
ACT = mybir.ActivationFunctionType

B, L, C = 1, 4096, 768
H, DK, DV = 4, 64, 192
POSF = 64
NQ = 2048          # queries per core (one of two blocks)
NT = 16            # query tiles of 128 per core
UW = 4223          # U window width per query tile
UP = 4224          # U row pitch in DRAM scratch
PKW = 6144         # per-core pos-key window (covers all 16 tiles)

_nc_cache = {}


def _build_nc():
    nc = bacc.Bacc()

    xt_in = nc.declare_dram_parameter("xt", (C, L), FP16, isOutput=False)
    xq_in = nc.declare_dram_parameter("xq", (C, NQ), FP16, isOutput=False)
    wq_in = nc.declare_dram_parameter("wq", (C, DK), FP16, isOutput=False)
    wk_in = nc.declare_dram_parameter("wk", (C, DK), FP16, isOutput=False)
    wv_in = nc.declare_dram_parameter("wv", (C, DV), FP16, isOutput=False)
    wpos_in = nc.declare_dram_parameter("wpos", (POSF, DK), FP16, isOutput=False)
    post_in = nc.declare_dram_parameter("post", (POSF, PKW), FP16, isOutput=False)
    wout_in = nc.declare_dram_parameter("wout", (DV, C), BF16, isOutput=False)
    rcb_in = nc.declare_dram_parameter("rcb", (DK, 1), F32, isOutput=False)
    rpb_in = nc.declare_dram_parameter("rpb", (DK, 1), F32, isOutput=False)
    ident_in = nc.declare_dram_parameter("ident", (128, 128), BF16, isOutput=False)
    out_dram = nc.declare_dram_parameter("out", (NQ, C), FP16, isOutput=True)

    with tile.TileContext(nc) as tc:
        with (
            tc.tile_pool(name="const", bufs=1) as cpool,
            tc.tile_pool(name="res", bufs=1) as rpool,
            tc.tile_pool(name="udram", bufs=3, space="DRAM") as dpool,
        ):
            # ---------- constants ----------
            wq_sb = cpool.tile([128, 6, DK], FP16)
            nc.gpsimd.dma_start(wq_sb[:], wq_in.rearrange("(cc p) d -> p cc d", p=128))
            wk_sb = cpool.tile([128, 6, DK], FP16)
            nc.gpsimd.dma_start(wk_sb[:], wk_in.rearrange("(cc p) d -> p cc d", p=128))
            wv_sb = cpool.tile([128, 6, DV], FP16)
            nc.gpsimd.dma_start(wv_sb[:], wv_in.rearrange("(cc p) d -> p cc d", p=128))
            wpos_sb = cpool.tile([POSF, DK], FP16)
            nc.gpsimd.dma_start(wpos_sb[:], wpos_in[:])
            wout1_sb = cpool.tile([128, C], BF16)
            nc.gpsimd.dma_start(wout1_sb[:], wout_in[0:128, :])
            wout2_sb = cpool.tile([64, C], BF16)
            nc.gpsimd.dma_start(wout2_sb[:], wout_in[128:192, :])
            rcb_sb = cpool.tile([DK, 1], F32)
            nc.gpsimd.dma_start(rcb_sb[:], rcb_in[:])
            rpb_sb = cpool.tile([DK, 1], F32)
            nc.gpsimd.dma_start(rpb_sb[:], rpb_in[:])
            ident_sb = cpool.tile([128, 128], BF16)
            nc.gpsimd.dma_start(ident_sb[:], ident_in[:])
            post_sb = cpool.tile([POSF, PKW], FP16)
            nc.gpsimd.dma_start(post_sb[:], post_in[:])

            # ---------- residents ----------
            xt_sb = rpool.tile([128, 6, L], FP16)      # x^T, full seq
            for cc in range(6):
                nc.sync.dma_start(xt_sb[:, cc, :],
                                  xt_in[cc * 128:(cc + 1) * 128, :])
            xq_sb = rpool.tile([128, 6, NQ], FP16)     # x^T, query block
            nc.sync.dma_start(xq_sb[:],
                              xq_in.rearrange("(cc p) d -> p cc d", p=128))
            kt_sb = rpool.tile([DK, L], FP16)          # K^T
            vp_sb = rpool.tile([128, 32, DV + 1], BF16)  # V per key blk + ones
            pkt_sb = rpool.tile([DK, PKW], FP16)       # pos_k^T window
            qct_sb = rpool.tile([DK, NQ], FP16)        # (Q/8 + rcb)^T
            qpt_sb = rpool.tile([DK, NQ], FP16)        # (Q/8 + rpb)^T

            nc.vector.memset(vp_sb[:, :, DV:DV + 1], 1.0)

            # ---------- phase A: projections ----------
            with (
                tc.tile_pool(name="psA", bufs=2, space="PSUM") as psA,
                tc.tile_pool(name="psV", bufs=2, space="PSUM") as psV,
                tc.tile_pool(name="psQ", bufs=2, space="PSUM") as psQ,
            ):
                # Q (+ biases, /8) -> qct, qpt
                for qc in range(4):
                    q_ps = psQ.tile([DK, 512], F32, tag="psq")
                    for cc in range(6):
                        nc.tensor.matmul(q_ps[:], wq_sb[:, cc, :],
                                         xq_sb[:, cc, qc * 512:(qc + 1) * 512],
                                         start=(cc == 0), stop=(cc == 5))
                    nc.scalar.activation(qct_sb[:, qc * 512:(qc + 1) * 512],
                                         q_ps[:], ACT.Identity,
                                         bias=rcb_sb[:], scale=0.125)
                    nc.scalar.activation(qpt_sb[:, qc * 512:(qc + 1) * 512],
                                         q_ps[:], ACT.Identity,
                                         bias=rpb_sb[:], scale=0.125)
                # pos_k^T window
                for mc in range(12):
                    pk_ps = psA.tile([DK, 512], F32, tag="psa")
                    nc.tensor.matmul(pk_ps[:], wpos_sb[:],
                                     post_sb[:, mc * 512:(mc + 1) * 512],
                                     start=True, stop=True)
                    if mc % 2 == 0:
                        nc.vector.tensor_copy(
                            pkt_sb[:, mc * 512:(mc + 1) * 512], pk_ps[:])
                    else:
                        nc.scalar.copy(
                            pkt_sb[:, mc * 512:(mc + 1) * 512], pk_ps[:])
                # K^T
                for kc in range(8):
                    kt_ps = psA.tile([DK, 512], F32, tag="psa")
                    for cc in range(6):
                        nc.tensor.matmul(kt_ps[:], wk_sb[:, cc, :],
                                         xt_sb[:, cc, kc * 512:(kc + 1) * 512],
                                         start=(cc == 0), stop=(cc == 5))
                    if kc % 2 == 0:
                        nc.vector.tensor_copy(
                            kt_sb[:, kc * 512:(kc + 1) * 512], kt_ps[:])
                    else:
                        nc.scalar.copy(
                            kt_sb[:, kc * 512:(kc + 1) * 512], kt_ps[:])
                # V (key-major, with ones column preset)
                for jb in range(32):
                    v_ps = psV.tile([128, DV], F32, tag="psv")
                    for cc in range(6):
                        nc.tensor.matmul(
                            v_ps[:], xt_sb[:, cc, jb * 128:(jb + 1) * 128],
                            wv_sb[:, cc, :], start=(cc == 0), stop=(cc == 5))
                    nc.vector.tensor_copy(vp_sb[:, jb, 0:DV], v_ps[:])

            # ---------- phase B: attention (software pipelined) ----------
            with (
                tc.tile_pool(name="ub", bufs=2) as upool,
                tc.tile_pool(name="usk", bufs=2) as uskpool,
                tc.tile_pool(name="at", bufs=2) as apool,
                tc.tile_pool(name="att", bufs=2) as atpool,
                tc.tile_pool(name="zz", bufs=4) as zpool,
                tc.tile_pool(name="ot", bufs=2) as opool,
                tc.tile_pool(name="fin", bufs=2) as fpool,
                tc.tile_pool(name="psU", bufs=2, space="PSUM") as psU,
                tc.tile_pool(name="psC", bufs=3, space="PSUM") as psC,
                tc.tile_pool(name="psT", bufs=1, space="PSUM") as psT,
                tc.tile_pool(name="psO", bufs=1, space="PSUM") as psO,
                tc.tile_pool(name="psP", bufs=1, space="PSUM") as psP,
            ):
                def emit_u(qt):
                    """U[p, m] = y_p . pkt[ws + m]  (width 4223), to DRAM."""
                    ws = 1920 - 128 * qt
                    u_sb = upool.tile([128, UP], FP16, tag="u")
                    for uc in range(9):
                        w = 512 if uc < 8 else UW - 8 * 512
                        u_ps = psU.tile([128, 512], F32, tag="psu")
                        nc.tensor.matmul(
                            u_ps[:, 0:w],
                            qpt_sb[:, qt * 128:(qt + 1) * 128],
                            pkt_sb[:, ws + uc * 512: ws + uc * 512 + w],
                            start=True, stop=True)
                        if uc % 2 == 0:
                            nc.vector.tensor_copy(
                                u_sb[:, uc * 512: uc * 512 + w], u_ps[:, 0:w])
                        else:
                            nc.scalar.copy(
                                u_sb[:, uc * 512: uc * 512 + w], u_ps[:, 0:w])
                    u_dr = dpool.tile([128, UP], FP16, tag="udr")
                    nc.sync.dma_start(u_dr[:], u_sb[:])
                    return u_dr

                def emit_skew(u_dr):
                    """usk[p, j] = U[p, j + 127 - p] via strided DRAM read."""
                    usk = uskpool.tile([128, L], FP16, tag="usk")
                    skew = bass.AP(u_dr.tensor, u_dr[:].offset + 127,
                                   [[UW, 128], [1, L]])
                    nc.gpsimd.dma_start(usk[:], skew)
                    return usk

                def emit_content(qt, usk, chunks):
                    """content matmul + usk add + exp for given 512-chunks."""
                    for ch in chunks:
                        c_ps = psC.tile([128, 512], F32, tag="psc")
                        nc.tensor.matmul(
                            c_ps[:], qct_sb[:, qt * 128:(qt + 1) * 128],
                            kt_sb[:, ch * 512:(ch + 1) * 512],
                            start=True, stop=True)
                        nc.vector.tensor_add(c_ps[:], c_ps[:],
                                             usk[:, ch * 512:(ch + 1) * 512])
                        nc.scalar.activation(
                            state[qt]["attn"][:, ch * 512:(ch + 1) * 512],
                            c_ps[:], ACT.Exp)

                def emit_transpose(qt):
                    attn = state[qt]["attn"]
                    attnT = atpool.tile([128, 32, 128], BF16, tag="attnT")
                    for g in range(8):
                        t_ps = psT.tile([128, 512], BF16, tag="pst")
                        for j in range(4):
                            kb = g * 4 + j
                            nc.tensor.transpose(
                                t_ps[:, j * 128:(j + 1) * 128],
                                attn[:, kb * 128:(kb + 1) * 128],
                                ident_sb[:])
                        nc.vector.tensor_copy(
                            attnT[:, g * 4:(g + 1) * 4, :],
                            t_ps[:].rearrange("p (j c) -> p j c", j=4))
                    state[qt]["attnT"] = attnT

                def emit_oacc_proj(qt):
                    attnT = state[qt]["attnT"]
                    o_ps = psO.tile([128, DV + 1], F32, tag="po")
                    for kb in range(32):
                        nc.tensor.matmul(o_ps[:], attnT[:, kb, :],
                                         vp_sb[:, kb, :],
                                         start=(kb == 0), stop=(kb == 31))
                    rz = zpool.tile([128, 1], F32, tag="rz")
                    nc.vector.reciprocal(rz[:], o_ps[:, DV:DV + 1])
                    o_sb = opool.tile([128, DV], BF16, tag="o")
                    nc.scalar.copy(o_sb[:], o_ps[:, 0:DV])
                    # transpose o -> oT (128+64 partitions)
                    t_ps = psT.tile([128, 512], BF16, tag="pst")
                    nc.tensor.transpose(t_ps[:, 0:128], o_sb[:, 0:128],
                                        ident_sb[:])
                    nc.tensor.transpose(t_ps[0:64, 128:256], o_sb[:, 128:192],
                                        ident_sb[:])
                    oT1 = opool.tile([128, 128], BF16, tag="oT1")
                    nc.vector.tensor_copy(oT1[:], t_ps[:, 0:128])
                    oT2 = opool.tile([64, 128], BF16, tag="oT2")
                    nc.vector.tensor_copy(oT2[:], t_ps[0:64, 128:256])
                    fin = fpool.tile([128, C], FP16, tag="fin")
                    for n0 in (0, 384):
                        p_ps = psP.tile([128, 384], F32, tag="pp")
                        nc.tensor.matmul(p_ps[:], oT1[:],
                                         wout1_sb[:, n0:n0 + 384],
                                         start=True, stop=False)
                        nc.tensor.matmul(p_ps[:], oT2[:],
                                         wout2_sb[:, n0:n0 + 384],
                                         start=False, stop=True)
                        nc.scalar.activation(fin[:, n0:n0 + 384], p_ps[:],
                                             ACT.Copy, scale=rz[:])
                    nc.gpsimd.dma_start(
                        out_dram[qt * 128:(qt + 1) * 128, :], fin[:])

                state = [dict() for _ in range(NT)]
                # prime the pipeline: U(0), U(1), skew(0)
                state[0]["udr"] = emit_u(0)
                state[1]["udr"] = emit_u(1)
                state[0]["usk"] = emit_skew(state[0]["udr"])

                for t in range(NT):
                    attn_t = apool.tile([128, L], BF16, tag="attn")
                    state[t]["attn"] = attn_t
                    # content chunks 0-2 (usk(t) already in flight/landed)
                    emit_content(t, state[t]["usk"], range(0, 3))
                    # U emit for t+2 (PE filler while exps of t catch up)
                    if t + 2 < NT:
                        state[t + 2]["udr"] = emit_u(t + 2)
                    if t + 1 < NT:
                        state[t + 1]["usk"] = emit_skew(state[t + 1]["udr"])
                    emit_content(t, state[t]["usk"], range(3, 6))
                    # o-acc + projection for t-1 (more PE filler)
                    if t > 0:
                        emit_oacc_proj(t - 1)
                    emit_content(t, state[t]["usk"], range(6, 8))
                    emit_transpose(t)
                emit_oacc_proj(NT - 1)

    nc.finalize()
    return nc


def _positions_T():
    feat = POSF // 2
    pow_rate = np.exp(np.log(L + 1) / feat).astype(np.float64)
    pos = np.arange(-L + 1, L, dtype=np.float64)                 # (8191,)
    cw = pow_rate ** np.arange(1, feat + 1, dtype=np.float64) - 1.0
    emb = (cw[None, :] > np.abs(pos)[:, None]).astype(np.float32)
    signed = np.sign(pos)[:, None].astype(np.float32) * emb
    p = np.concatenate([emb, signed], axis=-1)                   # (8191, 64)
    pt = np.zeros((POSF, 2 * L), np.float32)
    pt[:, :2 * L - 1] = p.T
    return pt


def kernel(x, Wq, Wk, Wv, Wpos, Wout, bout, rel_content_bias, rel_pos_bias):
    bf = ml_dtypes.bfloat16
    f16 = np.float16
    if "nc" not in _nc_cache:
        _nc_cache["nc"] = _build_nc()
    nc = _nc_cache["nc"]

    xt = np.ascontiguousarray(x[0].T).astype(f16)                 # (C, L)
    posT = _positions_T()                                        # (64, 8192)
    ident = np.eye(128, dtype=bf)

    in_maps = []
    for c in range(8):
        h, b = c // 2, c % 2
        w0 = 3968 - 2048 * b
        in_maps.append({
            "xt": xt,
            "xq": np.ascontiguousarray(x[0, b * NQ:(b + 1) * NQ].T).astype(f16),
            "wq": Wq[:, h * DK:(h + 1) * DK].astype(f16),
            "wk": Wk[:, h * DK:(h + 1) * DK].astype(f16),
            "wv": Wv[:, h * DV:(h + 1) * DV].astype(f16),
            "wpos": Wpos[:, h * DK:(h + 1) * DK].astype(f16),
            "post": np.ascontiguousarray(
                posT[:, w0 - 1920: w0 - 1920 + PKW]).astype(f16),
            "wout": Wout[h * DV:(h + 1) * DV, :].astype(bf),
            "rcb": np.ascontiguousarray(
                rel_content_bias[0, h, 0][:, None]).astype(np.float32),
            "rpb": np.ascontiguousarray(
                rel_pos_bias[0, h, 0][:, None]).astype(np.float32),
            "ident": ident,
        })

    res = run_bass_kernel_spmd(nc, in_maps, core_ids=list(range(8)))
    globals()["last_results"] = res
    parts = [r["out"] for r in res.results]

    out = np.zeros((L, C), np.float32)
    for b in range(2):
        acc = np.zeros((NQ, C), np.float32)
        for h in range(4):
            acc += parts[h * 2 + b].astype(np.float32)
        out[b * NQ:(b + 1) * NQ] = acc
    out += bout[None, :].astype(np.float32)
    return out.reshape(1, L, C)
